# revision 1
# baseline (speedup 1.0000x reference)
"""Trainium2 Bass kernel for nn_MoELayer (top-2 MoE, E=8 experts).

Strategy (expert-parallel across 8 NeuronCores):
  - Host computes the (tiny) gate matmul + top-2 + softmax, and dispatches
    each token to its two experts' cores ("all-to-all" done host-side as the
    sharding step). One expert per core.
  - Each core runs a Bass kernel computing, for its expert e and its routed
    tokens:   out = (silu(tok @ W1[e]) @ W2[e]) * gate_weight
    with bf16 matmul inputs and fp32 PSUM accumulation. Weights stay
    resident in SBUF; only the top-2-selected tokens are computed
    (4x fewer FLOPs than the dense reference).
  - Host scatter-adds the two weighted expert outputs per token.

Layouts (chosen so no on-device transposes are needed):
  stage 1:  actT[f, c] = silu( sum_d W1[d, f] * tokT[d, c] )
            matmul(lhsT=W1[dk, fj-tile], rhs=tokT[dk, c-chunk]) -> PSUM [f, c]
  stage 2:  out[c, d] = sum_f actT[f, c] * W2[f, d]
            matmul(lhsT=actT[fk, c-tile], rhs=W2[fk, d-chunk]) -> PSUM [c, d]

C (token capacity per core) is the exact max routed-token count, not
rounded up: stage 1 chunks may have non-multiple-of-128 widths and the
final stage-2 token tile may have <128 partitions.
"""

import math
import sys

sys.path.insert(0, "/opt/trn_rl_repo")

import ml_dtypes
import numpy as np

B, T, D, F, E = 2, 2048, 1024, 4096, 8
N = B * T
P = 128
KD = D // P  # 8
KF = F // P  # 32

bf16 = ml_dtypes.bfloat16

_nc_cache: dict[int, object] = {}
LAST_RESULTS = None  # BassKernelResults from the most recent run (for test.py)
TRACE = False


def _chunk_sizes(C: int) -> list[int]:
    """Split C into near-equal chunks of <=512 (stage-1 matmul free dim /
    PSUM bank limit), smallest first so the critical first token transfer
    is as small as possible."""
    n = math.ceil(C / 512)
    base = math.ceil(C / (n * P)) * P
    sizes = []
    rem = C
    while rem > 0:
        s = min(base, rem)
        sizes.append(s)
        rem -= s
    return sorted(sizes)


def _build(C: int):
    import concourse.mybir as mybir
    import concourse.tile as tile
    from concourse import bacc

    dt = mybir.dt

    nc = bacc.Bacc(None, target_bir_lowering=False)

    chunks = _chunk_sizes(C)

    # one token tensor per chunk -> fully contiguous per-partition DMA
    # packets (KD*cn*2 bytes) instead of 768B strided slices
    tokts = [
        nc.dram_tensor(f"tokt{i}", [P, KD, cn], dt.bfloat16, kind="ExternalInput")
        for i, cn in enumerate(chunks)
    ]
    w1 = nc.dram_tensor("w1", [P, KD, F], dt.bfloat16, kind="ExternalInput")
    w2 = nc.dram_tensor("w2", [P, KF, D], dt.bfloat16, kind="ExternalInput")
    # output is transposed: [D, C] with D on partitions; the gate-weight
    # scale + transpose happen on the host during scatter-add
    out = nc.dram_tensor("out", [D, C], dt.float32, kind="ExternalOutput")

    with tile.TileContext(nc) as tc:
        with (
            tc.tile_pool(name="const", bufs=1) as cpool,
            tc.tile_pool(name="act", bufs=1) as apool,
            tc.tile_pool(name="ps1", bufs=2, space="PSUM") as ps1pool,
            tc.tile_pool(name="ps2", bufs=2, space="PSUM") as ps2pool,
            tc.tile_pool(name="ob", bufs=4) as opool,
        ):
            w1_sb = cpool.tile([P, KD, F], dt.bfloat16, tag="w1")
            w2_sb = cpool.tile([P, KF, D], dt.bfloat16, tag="w2")
            tok_sbs = [
                cpool.tile(
                    [P, KD, cn], dt.bfloat16, tag=f"tok{i}", name=f"tok_sb{i}"
                )
                for i, cn in enumerate(chunks)
            ]

            # Input loads, all on the sync engine's HW DGE (SW DGE via other
            # engines measured far slower), emission-ordered by first use:
            # chunk-0 tokens, W1 quarters, remaining tokens, then W2.
            nc.sync.dma_start(tok_sbs[0][:], tokts[0][:])
            FQ = F // 4
            for q in range(4):
                nc.sync.dma_start(
                    w1_sb[:, :, q * FQ : (q + 1) * FQ],
                    w1[:, :, q * FQ : (q + 1) * FQ],
                )
            for i in range(1, len(chunks)):
                nc.sync.dma_start(tok_sbs[i][:], tokts[i][:])
            for q in range(4):
                nc.sync.dma_start(
                    w2_sb[:, q * (KF // 4) : (q + 1) * (KF // 4), :],
                    w2[:, q * (KF // 4) : (q + 1) * (KF // 4), :],
                )

            c0 = 0
            for ci, cn in enumerate(chunks):
                tok_sb = tok_sbs[ci]
                act_sb = apool.tile([P, KF, cn], dt.bfloat16, tag="act")
                # ---- stage 1: actT = silu(W1^T @ tokT) ----
                for fj in range(KF):
                    ps1 = ps1pool.tile([P, cn], dt.float32, tag="ps1")
                    for dk in range(KD):
                        nc.tensor.matmul(
                            ps1[:],
                            w1_sb[:, dk, fj * P : (fj + 1) * P],
                            tok_sb[:, dk, :],
                            start=(dk == 0),
                            stop=(dk == KD - 1),
                        )
                    nc.scalar.activation(
                        act_sb[:, fj, :],
                        ps1[:],
                        mybir.ActivationFunctionType.Silu,
                    )
                # ---- stage 2: outT = W2^T @ actT  (D on partitions,
                # tokens on the free dim -> no padded token tiles) ----
                for dm in range(D // P):
                    ps2 = ps2pool.tile([P, cn], dt.float32, tag="ps2")
                    for fk in range(KF):
                        nc.tensor.matmul(
                            ps2[:],
                            w2_sb[:, fk, dm * P : (dm + 1) * P],
                            act_sb[:, fk, :],
                            start=(fk == 0),
                            stop=(fk == KF - 1),
                        )
                    ob = opool.tile([P, cn], dt.float32, tag="ob")
                    nc.vector.tensor_copy(ob[:], ps2[:])
                    nc.sync.dma_start(
                        out[dm * P : (dm + 1) * P, c0 : c0 + cn],
                        ob[:],
                    )
                c0 += cn

    nc.compile()
    return nc


def _get_nc(C: int):
    if C not in _nc_cache:
        _nc_cache[C] = _build(C)
    return _nc_cache[C]


def kernel(**inputs) -> np.ndarray:
    global LAST_RESULTS
    x = np.asarray(inputs["x"], dtype=np.float32)
    Wg = np.asarray(inputs["Wg"], dtype=np.float32)
    W1 = np.asarray(inputs["W1"], dtype=np.float32)
    W2 = np.asarray(inputs["W2"], dtype=np.float32)

    h = np.ascontiguousarray(x.reshape(N, D))

    # ---- host gate: top-2 + softmax (0.05% of total FLOPs) ----
    logits = h @ Wg.T  # [N, E] f32
    idx2 = np.argpartition(-logits, 1, axis=1)[:, :2]
    lsel = np.take_along_axis(logits, idx2, axis=1)
    first = lsel[:, 0] >= lsel[:, 1]
    i0 = np.where(first, idx2[:, 0], idx2[:, 1])
    i1 = np.where(first, idx2[:, 1], idx2[:, 0])
    l0 = np.where(first, lsel[:, 0], lsel[:, 1])
    l1 = np.where(first, lsel[:, 1], lsel[:, 0])
    e1 = np.exp((l1 - l0).astype(np.float32))
    w0 = (1.0 / (1.0 + e1)).astype(np.float32)
    w1g = (e1 / (1.0 + e1)).astype(np.float32)

    token_ids = np.concatenate([np.arange(N), np.arange(N)])
    expert_ids = np.concatenate([i0, i1])
    gate_w = np.concatenate([w0, w1g])

    counts = np.bincount(expert_ids, minlength=E)
    C = int(counts.max())

    hb = h.astype(bf16)
    W1b = W1.astype(bf16)
    W2b = W2.astype(bf16)

    in_maps = []
    ids_per_expert = []
    gw_per_expert = []
    for e in range(E):
        sel = np.flatnonzero(expert_ids == e)
        ids_e = token_ids[sel]
        n_e = len(ids_e)
        ids_per_expert.append(ids_e)
        gw_per_expert.append(gate_w[sel])

        tokT = np.zeros((P, KD, C), dtype=bf16)
        # tokens [n,D] -> [D,n] -> [KD,P,n] -> [P,KD,n]
        tokT[:, :, :n_e] = (
            hb[ids_e].T.reshape(KD, P, n_e).transpose(1, 0, 2)
        )
        m = {
            "w1": np.ascontiguousarray(
                W1b[e].reshape(KD, P, F).transpose(1, 0, 2)
            ),
            "w2": np.ascontiguousarray(
                W2b[e].reshape(KF, P, D).transpose(1, 0, 2)
            ),
        }
        c0 = 0
        for i, cn in enumerate(_chunk_sizes(C)):
            m[f"tokt{i}"] = np.ascontiguousarray(tokT[:, :, c0 : c0 + cn])
            c0 += cn
        in_maps.append(m)

    nc = _get_nc(C)
    from concourse.bass_utils import run_bass_kernel_spmd

    LAST_RESULTS = run_bass_kernel_spmd(
        nc, in_maps, core_ids=list(range(E)), trace=TRACE
    )

    y = np.zeros((N, D), dtype=np.float32)
    for e in range(E):
        o = np.asarray(LAST_RESULTS.results[e]["out"], dtype=np.float32)  # [D, C]
        ids_e = ids_per_expert[e]
        n_e = len(ids_e)
        y[ids_e] += gw_per_expert[e][:, None] * o[:, :n_e].T
    return y.reshape(B, T, D)

